# revision 1
# baseline (speedup 1.0000x reference)
"""Dice loss kernel for Trainium2 (8 NeuronCores, SPMD data-parallel).

Problem: nn_DiceLoss — logits [8,19,512,512] f32, targets [8,512,512] int64.
  probs = softmax(logits, axis=1)
  PS[c] = sum_px probs[c,px]                  (probs_sum)
  I[c]  = sum_{px: t==c} probs[t(px),px]      (intersection)
  CT[c] = histogram(targets)                  (counts; host)
  dice  = (2I+1)/(PS+CT+1); loss = mean(1-dice)

Sharding: batch b -> core b.

Device computes the O(B*C*H*W) part: per-pixel softmax denominator S,
its reciprocal r (shipped back, bf16, 512KB/core), and the per-class
sums PS = sum_px probs. Host does the O(B*H*W) index work: gather the
true-class probability via r, I = bincount(t, weights=g), counts, and
the final dice — the same flavor of host-side index handling the
original version used to build one-hot masks, at 19x less data moved.

Each core's plane is viewed as [128 partitions, 2048 cols] and split in
4 column quarters; all input DMAs are contiguous 2D and issued up front.

Engine balance (per core, measured):
  ACT : exp for classes 0..12 from fp8 inputs (quantization noise
        washes out over the ~110k-element per-class sums), 2 chunked
        ops per quarter (4 for quarter 0), plus PSUM->SBUF staging
  DVE : exp for classes 13..18 via the Schraudolph bit trick (bf16
        bits of exp(l) ~ round(l*128/ln2 + 127*128), one tensor_scalar
        per quarter), reciprocal_approx_fast + bf16 cast, and
        W = E*r as ONE 2x-mode tensor_tensor per quarter with a
        stride-0 broadcast of r over classes   <- the dense spine,
        ~7.3us/quarter
  PE  : S = sum_c E_c (19 identity matmuls/quarter, FD 512, PSUM
        accumulation) and PS colsums via ones-column lhsT COL-TILED
        3-wide (classes c%3 land in 32-partition groups 0/32/64 and
        run concurrently in the array at ~50-120ns/matmul)
  DMA : fp8 logits for ACT classes + bf16 for DVE classes = 6.6MB in,
        3-6KB per-partition lines spread over all 16 SDMA engines

Quarter 3 is processed in two column halves to shorten the serial
S->recip->W->colsum tail. Found experimentally and kept out: W via a
fused scalar_tensor_tensor on bit patterns (no 2x uop: 8.3us/quarter),
casts on the scalar engine (queue behind exps delays the TT chain),
quarter-0 column halving and PE warmup bursts (cold-HAM small matmuls;
first ~9us is fixed DMA-start latency no matter the queue order).
"""

import sys

import numpy as np

sys.path.insert(0, "/opt/trn_rl_repo")

import ml_dtypes  # noqa: E402

B, C, H, W = 8, 19, 512, 512
HW = H * W  # 262144
P = 128  # partitions
NQ = 4  # column quarters
QC = 512  # columns per quarter
COLS = NQ * QC  # 2048 = HW / 128
NACT = 13  # classes 0..NACT-1 exp'd on ACT (fp8 input)
NSCH = C - NACT  # classes NACT..18 exp'd on DVE (bf16 input, Schraudolph)
FD8 = NACT * QC  # 6656
FDB = NSCH * QC  # 3072
GS = [7, 6, 6]  # colsum col-tile group sizes (class c -> group c%3, slot c//3)
A16 = 128.0 / float(np.log(2.0))  # Schraudolph scale for bf16 bit patterns
B16 = 127.0 * 128  # bf16 exponent bias in bit space
SMOOTH = 1.0
IGNORE_INDEX = 255

_CACHE = {}

# consts layout: identity [0:128], then per-class tiled ones-columns
_ONES_OFF = []
_off = 128
for _c in range(C):
    _ONES_OFF.append(_off)
    _off += GS[_c % 3]
CONST_COLS = _off


def _host_consts():
    bf16 = ml_dtypes.bfloat16
    cb = np.zeros((128, CONST_COLS), dtype=bf16)
    cb[:, 0:128] = np.eye(128, dtype=bf16)
    for c in range(C):
        cb[:, _ONES_OFF[c] + c // 3] = 1  # ones at this class's slot in its group
    return (cb,)


def _build_program():
    import concourse.bacc as bacc
    import concourse.mybir as mybir
    import concourse.tile as tile

    dt = mybir.dt
    AOP = mybir.AluOpType
    ACTF = mybir.ActivationFunctionType

    nc = bacc.Bacc("TRN2", target_bir_lowering=False, debug=False)
    x8_d = nc.declare_dram_parameter("x8", [NQ * P, FD8], dt.float8e4, isOutput=False)
    xb_d = nc.declare_dram_parameter("xb", [NQ * P, FDB], dt.bfloat16, isOutput=False)
    cb_d = nc.declare_dram_parameter(
        "consts_bf", [128, CONST_COLS], dt.bfloat16, isOutput=False
    )
    r_d = nc.declare_dram_parameter("r_out", [P, COLS], dt.bfloat16, isOutput=True)
    ps_d = nc.declare_dram_parameter("ps_out", [96, NQ * QC], dt.float32, isOutput=True)

    with tile.TileContext(nc) as tc:
        with (
            tc.tile_pool(name="singles", bufs=1) as sing,
            tc.tile_pool(name="X8p", bufs=4) as X8p,
            tc.tile_pool(name="Xbp", bufs=4) as Xbp,
            tc.tile_pool(name="Ep", bufs=3) as Ep,
            tc.tile_pool(name="Wp", bufs=2) as Wp,
            tc.tile_pool(name="Rfp", bufs=2) as Rfp,
            tc.tile_pool(name="Rbp", bufs=2) as Rbp,
            tc.tile_pool(name="psS", bufs=2, space="PSUM") as psS,
            tc.tile_pool(name="psAcc", bufs=1, space="PSUM") as psAcc,
        ):
            consts = sing.tile([128, CONST_COLS], dt.bfloat16)
            stage = sing.tile([96, NQ * QC], dt.float32)
            psPS = psAcc.tile([96, NQ * QC], dt.float32, tag="acc")
            ident = consts[0:128, 0:128]
            onescol = [
                consts[0:128, _ONES_OFF[c] : _ONES_OFF[c] + GS[c % 3]]
                for c in range(C)
            ]

            # prefetch everything up front; quarter 0 streamed in fine chunks
            X8s, Xbs = [], []
            X8s.append(X8p.tile([P, NACT, QC], dt.float8e4, tag="X8", name="X8t"))
            nc.gpsimd.dma_start(consts[:], cb_d[:])
            nc.sync.dma_start(X8s[0][:, 0:3, :], x8_d[0:P, 0 : 3 * QC])
            Xbs.append(Xbp.tile([P, NSCH, QC], dt.bfloat16, tag="Xb", name="Xbt"))
            nc.sync.dma_start(Xbs[0][:], xb_d[0:P, :])
            nc.sync.dma_start(X8s[0][:, 3:6, :], x8_d[0:P, 3 * QC : 6 * QC])
            nc.sync.dma_start(X8s[0][:, 6:9, :], x8_d[0:P, 6 * QC : 9 * QC])
            nc.sync.dma_start(X8s[0][:, 9:NACT, :], x8_d[0:P, 9 * QC :])
            for q in range(1, NQ):
                X8 = X8p.tile([P, NACT, QC], dt.float8e4, tag="X8", name="X8t")
                nc.sync.dma_start(X8[:], x8_d[P * q : P * (q + 1), :])
                X8s.append(X8)
                Xb = Xbp.tile([P, NSCH, QC], dt.bfloat16, tag="Xb", name="Xbt")
                nc.sync.dma_start(Xb[:], xb_d[P * q : P * (q + 1), :])
                Xbs.append(Xb)

            Es = []

            def emit_exp(q, chunks=((0, 7), (7, NACT))):
                """E[c] for all 19 classes: ACT exp (0..12) + DVE Schraudolph."""
                E = Ep.tile([P, C, QC], dt.bfloat16, tag="E", name="Et")
                nc.vector.tensor_scalar(
                    E[:, NACT:C, :].bitcast(dt.int16),
                    Xbs[q][:],
                    A16,
                    B16,
                    AOP.mult,
                    AOP.add,
                )
                for c0, c1 in chunks:
                    nc.scalar.activation(
                        E[:, c0:c1, :], X8s[q][:, c0:c1, :], ACTF.Exp
                    )
                Es.append(E)

            def emit_smm(q, SP, j0, j1):
                for c in range(C):
                    nc.tensor.matmul(
                        SP[:, 0 : j1 - j0],
                        ident,
                        Es[q][:, c, j0:j1],
                        start=(c == 0),
                        stop=(c == C - 1),
                    )

            def emit_recip(q, SP, j0, j1, Rb):
                Rf = Rfp.tile([P, QC], dt.float32, tag="Rf")
                nc.vector.reciprocal_approx_fast(Rf[:, 0 : j1 - j0], SP[:, 0 : j1 - j0])
                nc.vector.tensor_copy(Rb[:, j0:j1], Rf[:, 0 : j1 - j0])

            def emit_tt(q, Wt, Rb, j0, j1):
                """W = E * r, one 2x-mode tensor_tensor with broadcast r."""
                rb = Rb[:, j0:j1].unsqueeze(1).broadcast_to((P, C, j1 - j0))
                nc.vector.tensor_tensor(
                    out=Wt[:, :, j0:j1], in0=Es[q][:, :, j0:j1], in1=rb, op=AOP.mult
                )

            def emit_col(q, Wt, j0, j1):
                for c in range(C):
                    g = c % 3
                    nc.tensor.matmul(
                        psPS[32 * g : 32 * g + GS[g], QC * q + j0 : QC * q + j1],
                        onescol[c],
                        Wt[:, c, j0:j1],
                        start=(c < 3),
                        stop=(c >= C - 3),
                    )

            def emit_cps(q):
                # PSUM -> SBUF staging on the scalar engine (closest to PSUM)
                nc.scalar.copy(
                    stage[:, QC * q : QC * (q + 1)], psPS[0:96, QC * q : QC * (q + 1)]
                )
                nc.sync.dma_start(
                    ps_d[:, QC * q : QC * (q + 1)], stage[:, QC * q : QC * (q + 1)]
                )

            # ---- software-pipelined emission (engine FIFO order is the point)
            emit_exp(0, chunks=((0, 3), (3, 6), (6, 9), (9, NACT)))
            SP0 = psS.tile([P, QC], dt.float32, tag="S")
            emit_smm(0, SP0, 0, QC)
            emit_exp(1)
            Rb0 = Rbp.tile([P, QC], dt.bfloat16, tag="Rb")
            emit_recip(0, SP0, 0, QC, Rb0)
            nc.sync.dma_start(r_d[:, 0:QC], Rb0[:])
            W0 = Wp.tile([P, C, QC], dt.bfloat16, tag="W")
            emit_tt(0, W0, Rb0, 0, QC)
            SP1 = psS.tile([P, QC], dt.float32, tag="S")
            emit_smm(1, SP1, 0, QC)
            emit_col(0, W0, 0, QC)
            emit_exp(2)
            Rb1 = Rbp.tile([P, QC], dt.bfloat16, tag="Rb")
            emit_recip(1, SP1, 0, QC, Rb1)
            nc.sync.dma_start(r_d[:, QC : 2 * QC], Rb1[:])
            W1 = Wp.tile([P, C, QC], dt.bfloat16, tag="W")
            emit_tt(1, W1, Rb1, 0, QC)
            SP2 = psS.tile([P, QC], dt.float32, tag="S")
            emit_smm(2, SP2, 0, QC)
            emit_col(1, W1, 0, QC)
            emit_cps(0)
            emit_exp(3)
            Rb2 = Rbp.tile([P, QC], dt.bfloat16, tag="Rb")
            emit_recip(2, SP2, 0, QC, Rb2)
            nc.sync.dma_start(r_d[:, 2 * QC : 3 * QC], Rb2[:])
            W2 = Wp.tile([P, C, QC], dt.bfloat16, tag="W")
            emit_tt(2, W2, Rb2, 0, QC)
            # quarter 3 in two column halves to shorten the serial tail
            HC = QC // 2
            SP3a = psS.tile([P, QC], dt.float32, tag="S")
            emit_smm(3, SP3a, 0, HC)
            SP3b = psS.tile([P, QC], dt.float32, tag="S")
            emit_smm(3, SP3b, HC, QC)
            emit_col(2, W2, 0, QC)
            emit_cps(1)
            Rb3 = Rbp.tile([P, QC], dt.bfloat16, tag="Rb")
            W3 = Wp.tile([P, C, QC], dt.bfloat16, tag="W")
            emit_recip(3, SP3a, 0, HC, Rb3)
            emit_tt(3, W3, Rb3, 0, HC)
            emit_col(3, W3, 0, HC)
            emit_recip(3, SP3b, HC, QC, Rb3)
            emit_tt(3, W3, Rb3, HC, QC)
            nc.sync.dma_start(r_d[:, 3 * QC : 4 * QC], Rb3[:])
            emit_cps(2)
            nc.scalar.copy(
                stage[:, 3 * QC : 3 * QC + HC], psPS[0:96, 3 * QC : 3 * QC + HC]
            )
            nc.sync.dma_start(
                ps_d[:, 3 * QC : 3 * QC + HC], stage[:, 3 * QC : 3 * QC + HC]
            )
            emit_col(3, W3, HC, QC)
            nc.scalar.copy(stage[:, 3 * QC + HC :], psPS[0:96, 3 * QC + HC :])
            nc.sync.dma_start(ps_d[:, 3 * QC + HC :], stage[:, 3 * QC + HC :])

    nc.compile()
    return nc


def _get_program():
    if "nc" not in _CACHE:
        _CACHE["nc"] = _build_program()
        _CACHE["consts"] = _host_consts()
    return _CACHE["nc"], _CACHE["consts"]


def _install_ntff_hook():
    """antenv.axon_hooks is missing in this image; synthesize it so
    run_bass_kernel_spmd(trace=True) can capture NTFF profiles via axon."""
    import types

    if "antenv.axon_hooks" in sys.modules:
        return
    mod = types.ModuleType("antenv.axon_hooks")
    _h = [None]
    mod.set_axon_ntff_profile_hook = lambda h: _h.__setitem__(0, h)
    mod.get_axon_ntff_profile_hook = lambda: _h[0]
    sys.modules["antenv.axon_hooks"] = mod
    import antenv

    antenv.axon_hooks = mod
    from trn_agent_boot.trn_boot import _ntff_profile_via_ctypes

    mod.set_axon_ntff_profile_hook(
        _ntff_profile_via_ctypes("/opt/axon/libaxon_pjrt.so")
    )


def _prep_inputs(logits_np):
    """Quantize + re-lay out logits into per-core quarter tiles.

    Classes 0..12 as fp8 (ACT exp input), 13..18 as bf16 (DVE Schraudolph
    input). Quarter q of core b is a contiguous [128, Ccls*512] 2D block.
    """
    lg = np.asarray(logits_np, dtype=np.float32)
    l8 = lg[:, :NACT].astype(ml_dtypes.float8_e4m3fn)
    lb = lg[:, NACT:].astype(ml_dtypes.bfloat16)
    X8 = np.ascontiguousarray(
        l8.reshape(B, NACT, P, NQ, QC).transpose(0, 3, 2, 1, 4)
    ).reshape(B, NQ * P, FD8)
    Xb = np.ascontiguousarray(
        lb.reshape(B, NSCH, P, NQ, QC).transpose(0, 3, 2, 1, 4)
    ).reshape(B, NQ * P, FDB)
    return l8, lb, X8, Xb


def _run_device(logits_np, targets_np, trace=False):
    from concourse.bass_utils import run_bass_kernel_spmd

    nc, (cb,) = _get_program()
    l8, lb, X8, Xb = _prep_inputs(logits_np)
    in_maps = [{"x8": X8[b], "xb": Xb[b], "consts_bf": cb} for b in range(B)]
    kwargs = {}
    if trace:
        _install_ntff_hook()
        kwargs = {"trace": True, "trace_cores": [0]}
    res = run_bass_kernel_spmd(nc, in_maps, core_ids=list(range(B)), **kwargs)
    outs = [
        {
            "r_out": res.results[b]["r_out"],
            "ps_out": res.results[b]["ps_out"],
            "l8": l8[b],
            "lb": lb[b],
        }
        for b in range(B)
    ]
    return outs, res


def _ebits(l8b, lbb, cls, px):
    """int32 bf16-bit-patterns of E as the device computes them, for the
    given (class, pixel) index arrays."""
    bf16 = ml_dtypes.bfloat16
    out = np.empty(cls.shape, dtype=np.int32)
    act = cls < NACT
    if act.any():
        lv = l8b[cls[act], px[act]].astype(np.float32)
        out[act] = np.exp(lv).astype(bf16).view(np.int16)
    sch = ~act
    if sch.any():
        lv = lbb[cls[sch] - NACT, px[sch]].astype(np.float32)
        out[sch] = np.rint(lv * A16 + B16).astype(np.int16)
    return out


def _combine(outs, targets_np):
    bf16 = ml_dtypes.bfloat16
    t = np.asarray(targets_np).reshape(B, HW)
    PS = np.zeros(C, dtype=np.float64)
    I = np.zeros(C, dtype=np.float64)
    CT = np.zeros(C, dtype=np.float64)
    any_valid = False
    for b, o in enumerate(outs):
        st = o["ps_out"].astype(np.float64)  # [96, 2048] raw PSUM colsums
        for c in range(C):
            PS[c] += st[32 * (c % 3) + c // 3, :].sum()
        rvals = o["r_out"].reshape(HW).astype(np.float32)
        l8b = o["l8"].reshape(NACT, HW)
        lbb = o["lb"].reshape(NSCH, HW)
        tb = t[b]
        valid = tb != IGNORE_INDEX
        if not valid.any():
            continue
        any_valid = True
        tv = np.where(valid, tb, 0).astype(np.int64)
        px = np.arange(HW)
        eb = _ebits(l8b, lbb, tv, px)
        ev = eb.astype(np.int16).view(bf16).astype(np.float32)
        g = (ev * rvals).astype(bf16).astype(np.float64)
        I += np.bincount(tv[valid], weights=g[valid], minlength=C)
        CT += np.bincount(tv[valid], minlength=C)
        if not valid.all():
            inv = np.nonzero(~valid)[0]
            for c in range(C):
                eb = _ebits(l8b, lbb, np.full(len(inv), c), inv)
                ev = eb.astype(np.int16).view(bf16).astype(np.float32)
                PS[c] -= (ev * rvals[inv]).astype(bf16).astype(np.float64).sum()
    if not any_valid:
        return np.asarray(0.0, dtype=np.float32)
    dice = (2.0 * I + SMOOTH) / (PS + CT + SMOOTH)
    loss = (1.0 - dice).mean()
    return np.asarray(loss, dtype=np.float32)


def kernel(logits, targets):
    logits = np.asarray(logits)
    targets = np.asarray(targets)
    outs, _ = _run_device(logits, targets)
    return _combine(outs, targets)

